# revision 18
# baseline (speedup 1.0000x reference)
"""Multi-head attention (B=2, S=2048, D=1024, H=16, HD=64) on 8 trn2 cores.

Sharding: core c handles batch b = c // 4 and the 4 heads
[4*(c%4), 4*(c%4)+4)  (tensor-parallel split of the Wq/Wk/Wv column dim,
data-parallel over batch).  Each core computes its heads' full SxS
attention locally; no collectives.

v2 structure (ACT-exp is the roofline: 128 x [128,1024] exp instructions
~= 147us/core; everything else hides under it):
  1. X^T built in SBUF (bf16) via PE transposes; weights converted to bf16.
  2. Q^T/K^T (bf16, [dout, s]) and V (bf16, [s, dout] + ones column)
     computed with bf16 matmuls.
  3. Heads processed in PAIRS (even head on partitions 0-63, odd on
     64-127).  Per (pair, 512-wide m-chunk, t-tile): the two heads'
     score matmuls run CONCURRENTLY in separate PE row-groups
     (tile_position (0,0) / (64,0)) writing the two halves of one
     [128, 1024] PSUM tile; ONE exp instruction covers the pair; the two
     AV chains accumulate into the halves of one [65, 1024] PSUM tile
     (row 64 = softmax denominator via the V ones column).
  4. Software pipelining: scores(t+1) emitted before exp(t); one pending
     "filler" PE unit (j=1 projections, V chains, output transposes) is
     emitted per t-step so the PE gaps of the ACT-bound loop do useful
     work.  Output stage is deferred: only the av->SBUF copy is inline;
     transpose/normalize/DMA of m-chunk k run as fillers during chunk
     k+1.
"""

from collections import deque
from contextlib import ExitStack, nullcontext

import numpy as np

import concourse.bacc as bacc
import concourse.mybir as mybir
import concourse.tile as tile
from concourse.bass_utils import run_bass_kernel_spmd
from concourse.masks import make_identity

B, S, D = 2, 2048, 1024
H, HD = 16, 64
NCORES = 8
HPC = H * B // NCORES          # heads per core = 4
HG = HPC * HD                  # per-core projection width = 256
P = 128
KT = D // P                    # 8 contraction tiles
ST = S // P                    # 16 sequence tiles
MC = 512                       # m-chunk width for the attention loop
NMC = S // MC
VW = HD + 1                    # V columns per head incl. ones column = 65
NPAIR = HPC // 2               # head pairs per core = 2

F32 = mybir.dt.float32
BF16 = mybir.dt.bfloat16
F32R = mybir.dt.float32r
EXP = mybir.ActivationFunctionType.Exp


def _r(ap):
    return ap.bitcast(F32R)


def build_nc(reps=1):
    nc = bacc.Bacc(
        "TRN2", target_bir_lowering=False, debug=False, num_devices=NCORES
    )
    x = nc.dram_tensor("x", [S, D], F32, kind="ExternalInput")
    wq = nc.dram_tensor("wq", [D, HG], F32, kind="ExternalInput")
    wk = nc.dram_tensor("wk", [D, HG], F32, kind="ExternalInput")
    wv = nc.dram_tensor("wv", [D, HG], F32, kind="ExternalInput")
    # raw per-head AV blocks: row (h*VW + e) = head h, dim e (e==64 is the
    # softmax denominator); the divide + [e,s]->[s,e] transpose happen on the
    # host during unsharding
    out = nc.dram_tensor("out", [HPC * VW, S], F32, kind="ExternalOutput")

    with tile.TileContext(nc) as tc, ExitStack() as ctx:
        big = ctx.enter_context(tc.tile_pool(name="big", bufs=1))
        wstp = ctx.enter_context(tc.tile_pool(name="wstp", bufs=2))
        xst = ctx.enter_context(tc.tile_pool(name="xst", bufs=3))
        expp = ctx.enter_context(tc.tile_pool(name="expp", bufs=6))
        osbp = ctx.enter_context(tc.tile_pool(name="osbp", bufs=2))
        # PSUM budget (8 banks x 2KB): pp_sc 2 x [128,1024] f32 = 4 banks
        # (paired score tiles, double-buffered), pp_av 1 x [65,1024] = 2
        # banks (the pair's two AV accumulators), pp_ms 2 x [128,512] = 2
        # banks (X^T transposes, projection chunks, V chains, output
        # transposes).
        pp_sc = ctx.enter_context(tc.tile_pool(name="pp_sc", bufs=2, space="PSUM"))
        pp_av = ctx.enter_context(tc.tile_pool(name="pp_av", bufs=1, space="PSUM"))
        pp_ms = ctx.enter_context(tc.tile_pool(name="pp_ms", bufs=2, space="PSUM"))

        rep_ctx = tc.For_i(0, reps, 1) if reps > 1 else nullcontext()
        with rep_ctx:
            ident = big.tile([P, P], F32)
            make_identity(nc, ident[:])
            identr = big.tile([P, P], F32)
            nc.vector.tensor_copy(_r(identr[:]), ident[:])

            # ---- persistent SBUF tensors (bf16) ----
            XT = big.tile([P, KT * S], BF16)       # X^T: col(kt, s) = kt*S + s
            WQb = big.tile([P, KT * HG], BF16)     # col(kt, j) = kt*HG + j
            WKb = big.tile([P, KT * HG], BF16)
            WVb = big.tile([P, KT * HG], BF16)
            QT = big.tile([P, 2 * S], BF16)        # col(j, m) = j*S + m
            KTt = big.tile([P, 2 * S], BF16)
            Vn = big.tile([P, ST * HPC * VW], BF16)  # col(st,h,e) = st*HPC*VW + h*VW + e
            Vn4 = Vn[:].rearrange("p (s h e) -> p s h e", h=HPC, e=VW)

            # ---- load weights (one DMA each) and convert to bf16 ----
            for Wb, w in ((WQb, wq), (WKb, wk), (WVb, wv)):
                wst = wstp.tile([P, KT * HG], F32, tag="wst", name="wst")
                nc.sync.dma_start(
                    _r(wst[:].rearrange("p (k n) -> p k n", n=HG)),
                    _r(w[:].rearrange("(k p) n -> p k n", p=P)),
                )
                nc.vector.tensor_copy(Wb[:], wst[:])

            # ones columns of Vn (col 64 of each head block)
            ones_ap = Vn[:].rearrange("p (s h e) -> p s h e", h=HPC, e=VW)[
                :, :, :, HD:VW
            ]
            ones_stage = big.tile([P, ST * HPC], F32)
            nc.vector.memset(ones_stage[:], 1.0)
            nc.vector.tensor_copy(
                ones_ap,
                ones_stage[:].rearrange("p (s h e) -> p s h e", h=HPC, e=1),
            )

            # ---- load X (two s-tiles per DMA), PE-transpose into XT ----
            XT3 = XT[:].rearrange("p (k s) -> p k s", s=S)
            for sp in range(ST // 2):
                xs = xst.tile([P, 2 * D], F32, tag="xs", name="xs")
                nc.sync.dma_start(
                    _r(xs[:].rearrange("p (t d) -> p t d", d=D)),
                    _r(x[sp * 2 * P:(sp + 1) * 2 * P, :].rearrange(
                        "(t p) d -> p t d", p=P)),
                )
                for tt in range(2):
                    st_ = sp * 2 + tt
                    for g in range(2):
                        pt = pp_ms.tile([P, 512], F32, tag="ms", name="pt")
                        for jj in range(4):
                            kt = g * 4 + jj
                            nc.tensor.transpose(
                                _r(pt[:, jj * P:(jj + 1) * P]),
                                _r(xs[:, tt * D + kt * P: tt * D + (kt + 1) * P]),
                                _r(identr[:]),
                            )
                        # split psum->sbuf copies between DVE and the scalar
                        # engine (idle until the first exp)
                        if g == 0:
                            nc.vector.tensor_copy(
                                XT3[:, g * 4:(g + 1) * 4, st_ * P:(st_ + 1) * P],
                                pt[:].rearrange("p (k s) -> p k s", s=P),
                            )
                        else:
                            nc.scalar.copy(
                                XT3[:, g * 4:(g + 1) * 4, st_ * P:(st_ + 1) * P],
                                pt[:].rearrange("p (k s) -> p k s", s=P),
                            )

            # ---- PE work units ----
            def emit_proj_half(Wb, Ot, j, nn, half, state):
                """Half of a 512-col Q^T/K^T projection chunk (4 mm; the
                second half appends the psum->sbuf copy)."""
                if half == 0:
                    state["pt"] = pp_ms.tile([P, 512], F32, tag="ms", name="prj")
                pt = state["pt"]
                for kt in range(half * 4, half * 4 + 4):
                    nc.tensor.matmul(
                        pt[:],
                        Wb[:, kt * HG + j * P: kt * HG + (j + 1) * P],
                        XT[:, kt * S + nn * 512: kt * S + (nn + 1) * 512],
                        start=(kt == 0),
                        stop=(kt == KT - 1),
                    )
                if half == 1:
                    nc.vector.tensor_copy(
                        Ot[:, j * S + nn * 512: j * S + (nn + 1) * 512], pt[:]
                    )

            def emit_v_half(st_, half, state):
                """Half of a V chain for one s-tile (4 mm, all 4 heads)."""
                if half == 0:
                    state["pt"] = pp_ms.tile([P, 512], F32, tag="ms", name="vch")
                pt = state["pt"]
                for kt in range(half * 4, half * 4 + 4):
                    nc.tensor.matmul(
                        pt[:, 0:HG],
                        XT[:, kt * S + st_ * P: kt * S + (st_ + 1) * P],
                        WVb[:, kt * HG:(kt + 1) * HG],
                        start=(kt == 0),
                        stop=(kt == KT - 1),
                    )
                if half == 1:
                    nc.vector.tensor_copy(
                        Vn4[:, st_, :, 0:HD],
                        pt[:, 0:HG].rearrange("p (h e) -> p h e", e=HD),
                    )

            def emit_proj_chunk(Wb, Ot, j, nn):
                state = {}
                emit_proj_half(Wb, Ot, j, nn, 0, state)
                emit_proj_half(Wb, Ot, j, nn, 1, state)

            def emit_v_chain(st_):
                state = {}
                emit_v_half(st_, 0, state)
                emit_v_half(st_, 1, state)

            # ---- prologue PE work: Q^T/K^T for pair 0, V for s-tiles 0-9 ----
            for Wb, Ot in ((WQb, QT), (WKb, KTt)):
                for nn in range(4):
                    emit_proj_chunk(Wb, Ot, 0, nn)
            for st_ in range(10):
                emit_v_chain(st_)

            # ---- filler queues ----
            # dense: popped every step (V s-tiles 10-15, consumed at step
            # t==st of the first m-chunk).  sparse: popped every 3rd step so
            # the PE gap of the ACT-bound loop absorbs them (pair-1
            # projections, needed from step 64).
            dense = deque()
            sparse = deque()
            for st_ in range(10, ST):
                st8 = {}
                for half in range(2):
                    dense.append(
                        (lambda st_=st_, half=half, st8=st8:
                         emit_v_half(st_, half, st8))
                    )
            for Wb, Ot in ((WQb, QT), (WKb, KTt)):
                for nn in range(4):
                    st8 = {}
                    for half in range(2):
                        sparse.append(
                            (lambda Wb=Wb, Ot=Ot, nn=nn, half=half, st8=st8:
                             emit_proj_half(Wb, Ot, 1, nn, half, st8))
                        )

            # ---- attention: paired heads, ACT-bound inner loop ----
            def sc_pair(j, mc, t):
                """Both heads' score tiles for one t-tile, concurrently in
                PE row-groups 0-63 / 64-127 (tile_position auto-derived)."""
                ps = pp_sc.tile([P, 2 * MC], F32, tag="sc", name="ps")
                for hh in range(2):
                    nc.tensor.matmul(
                        ps[:, hh * MC:(hh + 1) * MC],
                        KTt[hh * 64:(hh + 1) * 64, j * S + t * P: j * S + (t + 1) * P],
                        QT[hh * 64:(hh + 1) * 64, j * S + mc * MC: j * S + (mc + 1) * MC],
                        start=True,
                        stop=True,
                    )
                return ps

            # flattened (pair, m-chunk, t) stream: the score lookahead crosses
            # chunk boundaries so the ACT engine never waits at them
            steps = [
                (j, mc, t)
                for j in range(NPAIR)
                for mc in range(NMC)
                for t in range(ST)
            ]
            # AV matmuls are emitted AVLAG steps late so a chunk-boundary
            # wait (on the av->osb evacuation) never head-blocks the PE FIFO
            # in front of the score matmuls the ACT engine needs next.
            AVLAG = 4
            avq = deque()

            def flush_av():
                j2, mc2, t2, ex2, av2 = avq.popleft()
                for hh in range(2):
                    h = 2 * j2 + hh
                    nc.tensor.matmul(
                        av2[:, hh * MC:(hh + 1) * MC],
                        Vn[:, t2 * HPC * VW + h * VW: t2 * HPC * VW + (h + 1) * VW],
                        ex2[:, hh * MC:(hh + 1) * MC],
                        start=(t2 == 0),
                        stop=(t2 == ST - 1),
                    )
                if t2 == ST - 1:
                    # evacuate the pair's AV accumulators and ship them raw
                    # (divide + transpose happen host-side during unshard)
                    osb = osbp.tile([VW, 2 * MC], F32, tag="osb", name="osb")
                    nc.vector.tensor_copy(osb[:], av2[:])
                    for hh in range(2):
                        h = 2 * j2 + hh
                        nc.sync.dma_start(
                            out[h * VW:(h + 1) * VW, mc2 * MC:(mc2 + 1) * MC],
                            osb[0:VW, hh * MC:(hh + 1) * MC],
                        )

            av = None
            ps_next = sc_pair(*steps[0])
            for k, (j, mc, t) in enumerate(steps):
                ps_cur = ps_next
                if t == 0:
                    av = pp_av.tile([VW, 2 * MC], F32, tag="av", name="av")
                if k + 1 < len(steps):
                    ps_next = sc_pair(*steps[k + 1])
                ex = expp.tile([P, 2 * MC], BF16, tag="ex", name="ex")
                nc.scalar.activation(
                    ex[:], ps_cur[:], EXP, scale=1.0 / np.sqrt(HD)
                )
                if dense:
                    dense.popleft()()
                elif sparse and k % 3 == 0:
                    sparse.popleft()()
                avq.append((j, mc, t, ex, av))
                if len(avq) > AVLAG:
                    flush_av()

            while avq:
                flush_av()
            while dense:
                dense.popleft()()
            while sparse:
                sparse.popleft()()

    nc.compile()
    return nc


_NC = None


def _get_nc():
    global _NC
    if _NC is None:
        _NC = build_nc()
    return _NC


def _shard_inputs(inputs, Wq, Wk, Wv):
    inputs = np.ascontiguousarray(np.asarray(inputs, dtype=np.float32))
    Wq = np.asarray(Wq, dtype=np.float32)
    Wk = np.asarray(Wk, dtype=np.float32)
    Wv = np.asarray(Wv, dtype=np.float32)
    in_maps = []
    for c in range(NCORES):
        b, g = c // (NCORES // B), c % (NCORES // B)
        sl = slice(g * HG, (g + 1) * HG)
        in_maps.append(
            {
                "x": inputs[b],
                "wq": np.ascontiguousarray(Wq[:, sl]),
                "wk": np.ascontiguousarray(Wk[:, sl]),
                "wv": np.ascontiguousarray(Wv[:, sl]),
            }
        )
    return in_maps


def _gather(results):
    out = np.empty((B, S, H * HD), dtype=np.float32)
    for c in range(NCORES):
        b, g = c // (NCORES // B), c % (NCORES // B)
        raw = results[c]["out"].reshape(HPC, VW, S)
        vals = raw[:, 0:HD, :] / raw[:, HD:VW, :]       # softmax normalize
        out[b, :, g * HG:(g + 1) * HG] = (
            vals.transpose(2, 0, 1).reshape(S, HG)
        )
    return out


def kernel(inputs, Wq, Wk, Wv):
    nc = _get_nc()
    in_maps = _shard_inputs(inputs, Wq, Wk, Wv)
    res = run_bass_kernel_spmd(nc, in_maps, core_ids=list(range(NCORES)))
    return _gather(res.results)


# revision 22
# speedup vs baseline: 1.0259x; 1.0259x over previous
"""Multi-head attention (B=2, S=2048, D=1024, H=16, HD=64) on 8 trn2 cores.

Sharding: core c handles batch b = c // 4 and the 4 heads
[4*(c%4), 4*(c%4)+4)  (tensor-parallel split of the Wq/Wk/Wv column dim,
data-parallel over batch).  Each core computes its heads' full SxS
attention locally; no collectives.

v2 structure (ACT-exp is the roofline: 128 x [128,1024] exp instructions
~= 147us/core; everything else hides under it):
  1. X^T built in SBUF (bf16) via PE transposes; weights converted to bf16.
  2. Q^T/K^T (bf16, [dout, s]) and V (bf16, [s, dout] + ones column)
     computed with bf16 matmuls.
  3. Heads processed in PAIRS (even head on partitions 0-63, odd on
     64-127).  Per (pair, 512-wide m-chunk, t-tile): the two heads'
     score matmuls run CONCURRENTLY in separate PE row-groups
     (tile_position (0,0) / (64,0)) writing the two halves of one
     [128, 1024] PSUM tile; ONE exp instruction covers the pair; the two
     AV chains accumulate into the halves of one [65, 1024] PSUM tile
     (row 64 = softmax denominator via the V ones column).
  4. Software pipelining: scores(t+1) emitted before exp(t); one pending
     "filler" PE unit (j=1 projections, V chains, output transposes) is
     emitted per t-step so the PE gaps of the ACT-bound loop do useful
     work.  Output stage is deferred: only the av->SBUF copy is inline;
     transpose/normalize/DMA of m-chunk k run as fillers during chunk
     k+1.
"""

from collections import deque
from contextlib import ExitStack, nullcontext

import numpy as np

import concourse.bacc as bacc
import concourse.mybir as mybir
import concourse.tile as tile
from concourse.bass_utils import run_bass_kernel_spmd
from concourse.masks import make_identity

B, S, D = 2, 2048, 1024
H, HD = 16, 64
NCORES = 8
HPC = H * B // NCORES          # heads per core = 4
HG = HPC * HD                  # per-core projection width = 256
P = 128
KT = D // P                    # 8 contraction tiles
ST = S // P                    # 16 sequence tiles
MC = 512                       # m-chunk width for the attention loop
NMC = S // MC
VW = HD + 1                    # V columns per head incl. ones column = 65
NPAIR = HPC // 2               # head pairs per core = 2

F32 = mybir.dt.float32
BF16 = mybir.dt.bfloat16
F32R = mybir.dt.float32r
EXP = mybir.ActivationFunctionType.Exp


def _r(ap):
    return ap.bitcast(F32R)


def build_nc(reps=1):
    nc = bacc.Bacc(
        "TRN2", target_bir_lowering=False, debug=False, num_devices=NCORES
    )
    x = nc.dram_tensor("x", [S, D], F32, kind="ExternalInput")
    wq = nc.dram_tensor("wq", [D, HG], F32, kind="ExternalInput")
    wk = nc.dram_tensor("wk", [D, HG], F32, kind="ExternalInput")
    wv = nc.dram_tensor("wv", [D, HG], F32, kind="ExternalInput")
    # raw per-head AV blocks: row (h*VW + e) = head h, dim e (e==64 is the
    # softmax denominator); the divide + [e,s]->[s,e] transpose happen on the
    # host during unsharding
    out = nc.dram_tensor("out", [HPC * VW, S], F32, kind="ExternalOutput")

    with tile.TileContext(nc) as tc, ExitStack() as ctx:
        big = ctx.enter_context(tc.tile_pool(name="big", bufs=1))
        wstp = ctx.enter_context(tc.tile_pool(name="wstp", bufs=2))
        xst = ctx.enter_context(tc.tile_pool(name="xst", bufs=8))
        expp = ctx.enter_context(tc.tile_pool(name="expp", bufs=5))
        osbp = ctx.enter_context(tc.tile_pool(name="osbp", bufs=2))
        # PSUM budget (8 banks x 2KB): pp_sc 2 x [128,1024] f32 = 4 banks
        # (paired score tiles, double-buffered), pp_av 1 x [65,1024] = 2
        # banks (the pair's two AV accumulators), pp_ms 2 x [128,512] = 2
        # banks (X^T transposes, projection chunks, V chains, output
        # transposes).
        pp_sc = ctx.enter_context(tc.tile_pool(name="pp_sc", bufs=2, space="PSUM"))
        pp_av = ctx.enter_context(tc.tile_pool(name="pp_av", bufs=1, space="PSUM"))
        pp_ms = ctx.enter_context(tc.tile_pool(name="pp_ms", bufs=2, space="PSUM"))

        rep_ctx = tc.For_i(0, reps, 1) if reps > 1 else nullcontext()
        with rep_ctx:
            ident = big.tile([P, P], F32)
            make_identity(nc, ident[:])
            identr = big.tile([P, P], F32)
            nc.vector.tensor_copy(_r(identr[:]), ident[:])

            # ---- persistent SBUF tensors (bf16) ----
            XT = big.tile([P, KT * S], BF16)       # X^T: col(kt, s) = kt*S + s
            WQb = big.tile([P, KT * HG], BF16)     # col(kt, j) = kt*HG + j
            WKb = big.tile([P, KT * HG], BF16)
            WVb = big.tile([P, KT * HG], BF16)
            QT = big.tile([P, 2 * S], BF16)        # col(j, m) = j*S + m
            KTt = big.tile([P, 2 * S], BF16)
            Vn = big.tile([P, ST * HPC * VW], BF16)  # col(st,h,e) = st*HPC*VW + h*VW + e
            Vn4 = Vn[:].rearrange("p (s h e) -> p s h e", h=HPC, e=VW)

            # ---- input DMAs, interleaved for just-in-time arrival: Wq
            # first (Q-proj is the first XT consumer), X chunks 0-1 (first
            # transposes), then Wk/Wv, then the rest of X ----
            XT3 = XT[:].rearrange("p (k s) -> p k s", s=S)
            xs_tiles = []

            def dma_x(sp):
                xs = xst.tile([P, 2 * D], F32, tag="xs", name="xs")
                nc.sync.dma_start(
                    _r(xs[:].rearrange("p (t d) -> p t d", d=D)),
                    _r(x[sp * 2 * P:(sp + 1) * 2 * P, :].rearrange(
                        "(t p) d -> p t d", p=P)),
                )
                xs_tiles.append(xs)

            def dma_w(Wb, w):
                wst = wstp.tile([P, KT * HG], F32, tag="wst", name="wst")
                nc.sync.dma_start(
                    _r(wst[:].rearrange("p (k n) -> p k n", n=HG)),
                    _r(w[:].rearrange("(k p) n -> p k n", p=P)),
                )
                nc.vector.tensor_copy(Wb[:], wst[:])

            dma_w(WQb, wq)
            dma_x(0)
            dma_x(1)
            dma_w(WKb, wk)
            dma_w(WVb, wv)
            for sp in range(2, ST // 2):
                dma_x(sp)

            # ones columns of Vn (col 64 of each head block)
            ones_ap = Vn[:].rearrange("p (s h e) -> p s h e", h=HPC, e=VW)[
                :, :, :, HD:VW
            ]
            ones_stage = big.tile([P, ST * HPC], F32)
            nc.vector.memset(ones_stage[:], 1.0)
            nc.vector.tensor_copy(
                ones_ap,
                ones_stage[:].rearrange("p (s h e) -> p s h e", h=HPC, e=1),
            )

            for sp in range(ST // 2):
                xs = xs_tiles[sp]
                for tt in range(2):
                    st_ = sp * 2 + tt
                    for g in range(2):
                        pt = pp_ms.tile([P, 512], F32, tag="ms", name="pt")
                        for jj in range(4):
                            kt = g * 4 + jj
                            nc.tensor.transpose(
                                _r(pt[:, jj * P:(jj + 1) * P]),
                                _r(xs[:, tt * D + kt * P: tt * D + (kt + 1) * P]),
                                _r(identr[:]),
                            )
                        # split psum->sbuf copies between DVE and the scalar
                        # engine (idle until the first exp)
                        if g == 0:
                            nc.vector.tensor_copy(
                                XT3[:, g * 4:(g + 1) * 4, st_ * P:(st_ + 1) * P],
                                pt[:].rearrange("p (k s) -> p k s", s=P),
                            )
                        else:
                            nc.scalar.copy(
                                XT3[:, g * 4:(g + 1) * 4, st_ * P:(st_ + 1) * P],
                                pt[:].rearrange("p (k s) -> p k s", s=P),
                            )

            # ---- PE work units ----
            def emit_proj_half(Wb, Ot, j, nn, half, state):
                """Half of a 512-col Q^T/K^T projection chunk (4 mm; the
                second half appends the psum->sbuf copy)."""
                if half == 0:
                    state["pt"] = pp_ms.tile([P, 512], F32, tag="ms", name="prj")
                pt = state["pt"]
                for kt in range(half * 4, half * 4 + 4):
                    nc.tensor.matmul(
                        pt[:],
                        Wb[:, kt * HG + j * P: kt * HG + (j + 1) * P],
                        XT[:, kt * S + nn * 512: kt * S + (nn + 1) * 512],
                        start=(kt == 0),
                        stop=(kt == KT - 1),
                    )
                if half == 1:
                    nc.vector.tensor_copy(
                        Ot[:, j * S + nn * 512: j * S + (nn + 1) * 512], pt[:]
                    )

            def emit_v_half(st_, half, state):
                """Half of a V chain for one s-tile (4 mm, all 4 heads)."""
                if half == 0:
                    state["pt"] = pp_ms.tile([P, 512], F32, tag="ms", name="vch")
                pt = state["pt"]
                for kt in range(half * 4, half * 4 + 4):
                    nc.tensor.matmul(
                        pt[:, 0:HG],
                        XT[:, kt * S + st_ * P: kt * S + (st_ + 1) * P],
                        WVb[:, kt * HG:(kt + 1) * HG],
                        start=(kt == 0),
                        stop=(kt == KT - 1),
                    )
                if half == 1:
                    nc.vector.tensor_copy(
                        Vn4[:, st_, :, 0:HD],
                        pt[:, 0:HG].rearrange("p (h e) -> p h e", e=HD),
                    )

            def emit_proj_chunk(Wb, Ot, j, nn):
                state = {}
                emit_proj_half(Wb, Ot, j, nn, 0, state)
                emit_proj_half(Wb, Ot, j, nn, 1, state)

            def emit_v_chain(st_):
                state = {}
                emit_v_half(st_, 0, state)
                emit_v_half(st_, 1, state)

            # ---- prologue PE work: Q^T/K^T for pair 0, V for s-tiles 0-9 ----
            for Wb, Ot in ((WQb, QT), (WKb, KTt)):
                for nn in range(4):
                    emit_proj_chunk(Wb, Ot, 0, nn)
            for st_ in range(10):
                emit_v_chain(st_)

            # ---- filler queues ----
            # dense: popped every step (V s-tiles 10-15, consumed at step
            # t==st of the first m-chunk).  sparse: popped every 3rd step so
            # the PE gap of the ACT-bound loop absorbs them (pair-1
            # projections, needed from step 64).
            dense = deque()
            sparse = deque()
            for st_ in range(10, ST):
                st8 = {}
                for half in range(2):
                    dense.append(
                        (lambda st_=st_, half=half, st8=st8:
                         emit_v_half(st_, half, st8))
                    )
            for Wb, Ot in ((WQb, QT), (WKb, KTt)):
                for nn in range(4):
                    st8 = {}
                    for half in range(2):
                        sparse.append(
                            (lambda Wb=Wb, Ot=Ot, nn=nn, half=half, st8=st8:
                             emit_proj_half(Wb, Ot, 1, nn, half, st8))
                        )

            # ---- attention: paired heads, ACT-bound inner loop ----
            def sc_pair(j, mc, t):
                """Both heads' score tiles for one t-tile, concurrently in
                PE row-groups 0-63 / 64-127 (tile_position auto-derived)."""
                ps = pp_sc.tile([P, 2 * MC], F32, tag="sc", name="ps")
                for hh in range(2):
                    nc.tensor.matmul(
                        ps[:, hh * MC:(hh + 1) * MC],
                        KTt[hh * 64:(hh + 1) * 64, j * S + t * P: j * S + (t + 1) * P],
                        QT[hh * 64:(hh + 1) * 64, j * S + mc * MC: j * S + (mc + 1) * MC],
                        start=True,
                        stop=True,
                    )
                return ps

            # flattened (pair, m-chunk, t) stream: the score lookahead crosses
            # chunk boundaries so the ACT engine never waits at them
            steps = [
                (j, mc, t)
                for j in range(NPAIR)
                for mc in range(NMC)
                for t in range(ST)
            ]
            # AV matmuls are emitted AVLAG steps late so a chunk-boundary
            # wait (on the av->osb evacuation) never head-blocks the PE FIFO
            # in front of the score matmuls the ACT engine needs next.
            AVLAG = 3
            avq = deque()

            def flush_av():
                j2, mc2, t2, ex2, av2 = avq.popleft()
                for hh in range(2):
                    h = 2 * j2 + hh
                    nc.tensor.matmul(
                        av2[:, hh * MC:(hh + 1) * MC],
                        Vn[:, t2 * HPC * VW + h * VW: t2 * HPC * VW + (h + 1) * VW],
                        ex2[:, hh * MC:(hh + 1) * MC],
                        start=(t2 == 0),
                        stop=(t2 == ST - 1),
                    )
                if t2 == ST - 1:
                    # evacuate the pair's AV accumulators and ship them raw
                    # (divide + transpose happen host-side during unshard)
                    osb = osbp.tile([VW, 2 * MC], F32, tag="osb", name="osb")
                    nc.vector.tensor_copy(osb[:], av2[:])
                    for hh in range(2):
                        h = 2 * j2 + hh
                        nc.sync.dma_start(
                            out[h * VW:(h + 1) * VW, mc2 * MC:(mc2 + 1) * MC],
                            osb[0:VW, hh * MC:(hh + 1) * MC],
                        )

            av = None
            ps_next = sc_pair(*steps[0])
            for k, (j, mc, t) in enumerate(steps):
                ps_cur = ps_next
                if t == 0:
                    av = pp_av.tile([VW, 2 * MC], F32, tag="av", name="av")
                if k + 1 < len(steps):
                    ps_next = sc_pair(*steps[k + 1])
                ex = expp.tile([P, 2 * MC], BF16, tag="ex", name="ex")
                nc.scalar.activation(
                    ex[:], ps_cur[:], EXP, scale=1.0 / np.sqrt(HD)
                )
                if dense:
                    dense.popleft()()
                elif sparse and k % 3 == 0:
                    sparse.popleft()()
                avq.append((j, mc, t, ex, av))
                if len(avq) > AVLAG:
                    flush_av()

            while avq:
                flush_av()
            while dense:
                dense.popleft()()
            while sparse:
                sparse.popleft()()

    nc.compile()
    return nc


_NC = None


def _get_nc():
    global _NC
    if _NC is None:
        _NC = build_nc()
    return _NC


def _shard_inputs(inputs, Wq, Wk, Wv):
    inputs = np.ascontiguousarray(np.asarray(inputs, dtype=np.float32))
    Wq = np.asarray(Wq, dtype=np.float32)
    Wk = np.asarray(Wk, dtype=np.float32)
    Wv = np.asarray(Wv, dtype=np.float32)
    in_maps = []
    for c in range(NCORES):
        b, g = c // (NCORES // B), c % (NCORES // B)
        sl = slice(g * HG, (g + 1) * HG)
        in_maps.append(
            {
                "x": inputs[b],
                "wq": np.ascontiguousarray(Wq[:, sl]),
                "wk": np.ascontiguousarray(Wk[:, sl]),
                "wv": np.ascontiguousarray(Wv[:, sl]),
            }
        )
    return in_maps


def _gather(results):
    out = np.empty((B, S, H * HD), dtype=np.float32)
    for c in range(NCORES):
        b, g = c // (NCORES // B), c % (NCORES // B)
        raw = results[c]["out"].reshape(HPC, VW, S)
        vals = raw[:, 0:HD, :] / raw[:, HD:VW, :]       # softmax normalize
        out[b, :, g * HG:(g + 1) * HG] = (
            vals.transpose(2, 0, 1).reshape(S, HG)
        )
    return out


def kernel(inputs, Wq, Wk, Wv):
    nc = _get_nc()
    in_maps = _shard_inputs(inputs, Wq, Wk, Wv)
    res = run_bass_kernel_spmd(nc, in_maps, core_ids=list(range(NCORES)))
    return _gather(res.results)
